# revision 1
# baseline (speedup 1.0000x reference)
"""Trainium2 Bass kernel for a Swin transformer block (shifted-window attention).

Self-contained: host-side does the roll+window permutation, folds LN/bias into
weight matrices, precomputes combined (rel-pos bias + shift mask) tables, and
shards 32 windows per NeuronCore (data-parallel, zero cross-core comm).
Device program per core: LN1 -> PE-transpose -> qk^T/v matmuls -> per-head
scores^T (row-packed matmuls) -> fused bias+mask add -> exp (max-free softmax,
unnormalized) -> attn@v + denominators via ones-matmul (col-packed) ->
normalize -> proj -> residual -> LN2 -> MLP(gelu) -> residual.
"""

import os
import sys
from contextlib import ExitStack

_REPO = "/opt/trn_rl_repo"
if _REPO not in sys.path:
    sys.path.insert(0, _REPO)

import numpy as np
import ml_dtypes

import concourse.mybir as mybir
import concourse.tile as tile
from concourse.masks import make_identity

F32 = mybir.dt.float32
BF16 = mybir.dt.bfloat16
AF = mybir.ActivationFunctionType

H = 256
W = 256
C = 192
HEADS = 6
WS = 16
SHIFT = 8
N = WS * WS
HD = C // HEADS
NW = (H // WS) * (W // WS)
HID = 4 * C
SCALE = HD ** -0.5
EPS = 1e-5
NCORES = 8
S = NW // NCORES          # 32 window slots per core
PAIRS = S // 2


# ---------------------------------------------------------------- host side

def _window_perm():
    idx = np.arange(H * W).reshape(H, W)
    idx_shift = np.roll(idx, (-SHIFT, -SHIFT), axis=(0, 1))
    return idx_shift.reshape(H // WS, WS, W // WS, WS).transpose(0, 2, 1, 3).reshape(NW, N)


def _fold_weights(norm1_g, norm1_b, qkv_w, qkv_b, proj_w, proj_b,
                  norm2_g, norm2_b, fc1_w, fc1_b, fc2_w, fc2_b):
    f32 = np.float32
    Wg = np.asarray(qkv_w, f32) * np.asarray(norm1_g, f32)[:, None]
    B = np.asarray(norm1_b, f32) @ np.asarray(qkv_w, f32) + np.asarray(qkv_b, f32)
    Wg[:, :C] *= SCALE
    B[:C] *= SCALE
    qk_cols = list(range(0, 128)) + list(range(C, C + 128)) + \
        list(range(128, C)) + list(range(C + 128, 2 * C))
    Wqk = Wg[:, qk_cols]
    Bqk = B[qk_cols]
    out = {}
    out["wqk_a"] = np.ascontiguousarray(Wqk[:128])
    out["wqk_b"] = np.ascontiguousarray(np.concatenate([Wqk[128:], Bqk[None]], 0))
    Wv = Wg[:, 2 * C:]
    Bv = B[2 * C:]
    out["wv_a"] = np.ascontiguousarray(Wv[:128])
    out["wv_b"] = np.ascontiguousarray(np.concatenate([Wv[128:], Bv[None]], 0))
    pw = np.asarray(proj_w, f32)
    out["wpj_a"] = np.ascontiguousarray(pw[:128])
    out["wpj_b"] = np.ascontiguousarray(
        np.concatenate([pw[128:], np.asarray(proj_b, f32)[None]], 0))
    Wg2 = np.asarray(fc1_w, f32) * np.asarray(norm2_g, f32)[:, None]
    B2 = np.asarray(norm2_b, f32) @ np.asarray(fc1_w, f32) + np.asarray(fc1_b, f32)
    out["wf1_a"] = np.ascontiguousarray(Wg2[:128])
    out["wf1_b"] = np.ascontiguousarray(np.concatenate([Wg2[128:], B2[None]], 0))
    out["wf2"] = np.ascontiguousarray(np.asarray(fc2_w, f32))
    fc2_b = np.asarray(fc2_b, f32)
    out["has_fc2_bias"] = bool(np.any(fc2_b != 0))
    if out["has_fc2_bias"]:
        out["wf2b"] = fc2_b[None]
    return out


def _build_cb_core(rpi_sa, rpb_table, attn_mask, windows):
    """[pairs, 128, 2slot, 6h, 2chunk, 256q] bf16 combined bias+mask (transposed)."""
    f32 = np.float32
    rpi = np.asarray(rpi_sa).reshape(-1).astype(np.int64)
    bias = np.asarray(rpb_table, f32)[rpi].reshape(N, N, HEADS)
    biasT = bias.transpose(2, 1, 0)                       # [h, k, q]
    maskT = np.asarray(attn_mask, f32).transpose(0, 2, 1)  # [w, k, q]
    wlist = list(windows)
    cbw = biasT[None] + maskT[wlist][:, None]              # [S, 6, k, q]
    cbw = cbw.reshape(len(wlist) // 2, 2, HEADS, 2, 128, N)
    cbw = cbw.transpose(0, 4, 1, 2, 3, 5)
    return np.ascontiguousarray(cbw.astype(ml_dtypes.bfloat16))


# ---------------------------------------------------------------- device side

def _build_nc(has_fc2_bias, num_devices=NCORES, repeat=1):
    import concourse.bacc as bacc
    nc = bacc.Bacc("TRN2", target_bir_lowering=False, debug=False,
                   num_devices=num_devices)
    dt = nc.dram_tensor
    tens = {}
    tens["xw"] = dt("xw", [S * N, C], F32, kind="ExternalInput").ap()
    tens["cb"] = dt("cb", [PAIRS, 128, 2, HEADS, 2, N], BF16,
                    kind="ExternalInput").ap()
    tens["wqk_a"] = dt("wqk_a", [128, 2 * C], F32, kind="ExternalInput").ap()
    tens["wqk_b"] = dt("wqk_b", [65, 2 * C], F32, kind="ExternalInput").ap()
    tens["wv_a"] = dt("wv_a", [128, C], F32, kind="ExternalInput").ap()
    tens["wv_b"] = dt("wv_b", [65, C], F32, kind="ExternalInput").ap()
    tens["wpj_a"] = dt("wpj_a", [128, C], F32, kind="ExternalInput").ap()
    tens["wpj_b"] = dt("wpj_b", [65, C], F32, kind="ExternalInput").ap()
    tens["wf1_a"] = dt("wf1_a", [128, HID], F32, kind="ExternalInput").ap()
    tens["wf1_b"] = dt("wf1_b", [65, HID], F32, kind="ExternalInput").ap()
    tens["wf2"] = dt("wf2", [HID, C], F32, kind="ExternalInput").ap()
    if has_fc2_bias:
        tens["wf2b"] = dt("wf2b", [1, C], F32, kind="ExternalInput").ap()
    tens["out"] = dt("out", [S * N, C], F32, kind="ExternalOutput").ap()
    tens["has_fc2_bias"] = has_fc2_bias
    tens["repeat"] = repeat
    with tile.TileContext(nc) as tc, ExitStack() as ctx:
        _build_body(ctx, tc, nc, tens)
    nc.compile()
    return nc


def _build_body(ctx, tc, nc, t):
    has_fc2_bias = t["has_fc2_bias"]
    xw, cb, out = t["xw"], t["cb"], t["out"]

    const = ctx.enter_context(tc.tile_pool(name="const", bufs=1))
    ps_tr = ctx.enter_context(tc.tile_pool(name="ps_tr", bufs=2, space="PSUM"))
    ps_big = ctx.enter_context(tc.tile_pool(name="ps_big", bufs=4, space="PSUM"))
    ps_ot = ctx.enter_context(tc.tile_pool(name="ps_ot", bufs=1, space="PSUM"))
    ps_ot2 = ctx.enter_context(tc.tile_pool(name="ps_ot2", bufs=1, space="PSUM"))
    sb_x = ctx.enter_context(tc.tile_pool(name="sb_x", bufs=10))
    sb_h = ctx.enter_context(tc.tile_pool(name="sb_h", bufs=6))
    sb_hT = ctx.enter_context(tc.tile_pool(name="sb_hT", bufs=3))
    sb_qk = ctx.enter_context(tc.tile_pool(name="sb_qk", bufs=3))
    sb_v = ctx.enter_context(tc.tile_pool(name="sb_v", bufs=10))
    sb_u = ctx.enter_context(tc.tile_pool(name="sb_u", bufs=6))
    sb_e = ctx.enter_context(tc.tile_pool(name="sb_e", bufs=15))
    sb_rd = ctx.enter_context(tc.tile_pool(name="sb_rd", bufs=4))
    sb_oT = ctx.enter_context(tc.tile_pool(name="sb_oT", bufs=3))
    sb_xr = ctx.enter_context(tc.tile_pool(name="sb_xr", bufs=10))
    sb_g = ctx.enter_context(tc.tile_pool(name="sb_g", bufs=14))
    sb_cb = ctx.enter_context(tc.tile_pool(name="sb_cb", bufs=2))
    sb_fin = ctx.enter_context(tc.tile_pool(name="sb_fin", bufs=6))
    sb_st = ctx.enter_context(tc.tile_pool(name="sb_st", bufs=24))

    ident = const.tile([128, 128], F32)
    make_identity(nc, ident)
    eps_t = const.tile([128, 1], F32)
    nc.vector.memset(eps_t, EPS)
    ones_blk = const.tile([128, 32], BF16)
    nc.vector.memset(ones_blk, 1.0)

    w_qk_a = const.tile([128, 2 * C], F32)
    nc.sync.dma_start(out=w_qk_a, in_=t["wqk_a"])
    w_qk_b = const.tile([65, 2 * C], F32)
    nc.sync.dma_start(out=w_qk_b, in_=t["wqk_b"])
    w_v_a = const.tile([128, C], F32)
    nc.sync.dma_start(out=w_v_a, in_=t["wv_a"])
    w_v_b = const.tile([65, C], F32)
    nc.sync.dma_start(out=w_v_b, in_=t["wv_b"])
    w_pj_a = const.tile([128, C], F32)
    nc.sync.dma_start(out=w_pj_a, in_=t["wpj_a"])
    w_pj_b = const.tile([65, C], F32)
    nc.sync.dma_start(out=w_pj_b, in_=t["wpj_b"])
    w_f1_a = const.tile([128, HID], F32)
    nc.sync.dma_start(out=w_f1_a, in_=t["wf1_a"])
    w_f1_b = const.tile([65, HID], F32)
    nc.sync.dma_start(out=w_f1_b, in_=t["wf1_b"])
    w_f2 = const.tile([128, 6, C], F32)
    nc.sync.dma_start(out=w_f2, in_=t["wf2"].rearrange("(c p) n -> p c n", p=128))
    if has_fc2_bias:
        w_f2b = const.tile([1, C], F32)
        nc.sync.dma_start(out=w_f2b, in_=t["wf2b"])
        ones_row = const.tile([1, 512], F32)
        nc.vector.memset(ones_row, 1.0)

    for p_ in range(PAIRS * t["repeat"]):
        p = p_ % PAIRS
        tok0 = p * 2 * N

        cb_t = sb_cb.tile([128, 2, HEADS, 2, N], BF16, tag="cb")
        nc.sync.dma_start(out=cb_t, in_=cb[p])

        # stage A: load + LN1 + transpose
        x_ts = []
        hT_a = sb_hT.tile([128, 512], F32, tag="hta")
        hT_b = sb_hT.tile([65, 512], F32, tag="htb")
        nc.gpsimd.memset(hT_b[64:65, :], 1.0)
        for i in range(4):
            x_t = sb_x.tile([128, C], F32, tag="x")
            nc.sync.dma_start(out=x_t,
                              in_=xw[tok0 + i * 128: tok0 + (i + 1) * 128, :])
            x_ts.append(x_t)
            st = sb_st.tile([128, 6], F32, tag="st")
            nc.vector.bn_stats(out=st, in_=x_t)
            mv = sb_st.tile([128, 2], F32, tag="mv")
            nc.vector.bn_aggr(out=mv, in_=st)
            sd = sb_st.tile([128, 1], F32, tag="sd")
            nc.scalar.activation(out=sd, in_=mv[:, 1:2], func=AF.Sqrt, bias=eps_t)
            rstd = sb_st.tile([128, 1], F32, tag="rstd")
            nc.vector.reciprocal(out=rstd, in_=sd)
            h_t = sb_h.tile([128, C], F32, tag="h")
            nc.gpsimd.tensor_scalar(out=h_t, in0=x_t, scalar1=mv[:, 0:1],
                                    scalar2=rstd, op0=mybir.AluOpType.subtract,
                                    op1=mybir.AluOpType.mult)
            ptr = ps_tr.tile([128, 256], F32, tag="tr")
            nc.tensor.transpose(ptr[:, 0:128], h_t[:, 0:128], ident)
            nc.tensor.transpose(ptr[0:64, 128:256], h_t[:, 128:192], ident)
            nc.scalar.copy(out=hT_a[:, i * 128:(i + 1) * 128], in_=ptr[:, 0:128])
            nc.scalar.copy(out=hT_b[0:64, i * 128:(i + 1) * 128],
                           in_=ptr[0:64, 128:256])

        # stage B: qk^T matmuls
        tq = sb_qk.tile([128, 512], F32, tag="tq")
        tk = sb_qk.tile([128, 512], F32, tag="tk")
        tq2 = sb_qk.tile([64, 512], F32, tag="tq2")
        tk2 = sb_qk.tile([64, 512], F32, tag="tk2")
        for (sl, dst, eng) in ((slice(0, 128), tq, "act"),
                               (slice(128, 256), tk, "act"),
                               (slice(256, 320), tq2, "dve"),
                               (slice(320, 384), tk2, "dve")):
            msz = sl.stop - sl.start
            pqk = ps_big.tile([128, 512], F32, tag="big")
            nc.tensor.matmul(pqk[0:msz, :], w_qk_a[:, sl], hT_a,
                             start=True, stop=False)
            nc.tensor.matmul(pqk[0:msz, :], w_qk_b[:, sl], hT_b,
                             start=False, stop=True)
            if eng == "act":
                nc.scalar.copy(out=dst, in_=pqk[0:msz, :])
            else:
                nc.vector.tensor_copy(out=dst, in_=pqk[0:msz, :])

        # stage Bv: v matmuls (token rows)
        tvs = []
        for i in range(4):
            pv = ps_tr.tile([128, C], F32, tag="tr")
            nc.tensor.matmul(pv, hT_a[:, i * 128:(i + 1) * 128], w_v_a,
                             start=True, stop=False)
            nc.tensor.matmul(pv, hT_b[:, i * 128:(i + 1) * 128], w_v_b,
                             start=False, stop=True)
            tv = sb_v.tile([128, C], BF16, tag="tv")
            nc.scalar.copy(out=tv, in_=pv)
            tvs.append(tv)

        # stage C: scores^T + bias/mask + exp
        e_ts = [[None] * 2 for _ in range(HEADS)]
        for c_ in range(2):
            for h in range(HEADS):
                if h < 4:
                    ktile, qtile, b0 = tk, tq, 32 * h
                else:
                    ktile, qtile, b0 = tk2, tq2, 32 * (h - 4)
                s_ps = ps_big.tile([128, 512], F32, tag="big")
                for w_ in range(2):
                    kcols = slice(w_ * 256 + c_ * 128, w_ * 256 + c_ * 128 + 128)
                    qcols = slice(w_ * 256, (w_ + 1) * 256)
                    nc.tensor.matmul(
                        s_ps[:, w_ * 256:(w_ + 1) * 256],
                        ktile[b0:b0 + 32, kcols],
                        qtile[b0:b0 + 32, qcols],
                        start=True, stop=True, tile_position=(b0, 0))
                u_t = sb_u.tile([128, 512], BF16, tag="u")
                nc.vector.tensor_tensor(out=u_t, in0=s_ps,
                                        in1=cb_t[:, :, h, c_, :],
                                        op=mybir.AluOpType.add)
                e_t = sb_e.tile([128, 512], BF16, tag="e")
                nc.scalar.activation(out=e_t, in_=u_t, func=AF.Exp)
                e_ts[h][c_] = e_t

        # stage D: attn@v (transposed) + denominators, then normalize
        oT_a = sb_oT.tile([128, 512], F32, tag="ota")
        oT_b = sb_oT.tile([65, 512], F32, tag="otb")
        nc.gpsimd.memset(oT_b[64:65, :], 1.0)
        for w_ in range(2):
            ot = ps_ot.tile([128, 512], F32, tag="ot")
            ot2 = ps_ot2.tile([64, 512], F32, tag="ot2")
            for h in range(HEADS):
                if h < 4:
                    dst, b0 = ot, 32 * h
                else:
                    dst, b0 = ot2, 32 * (h - 4)
                for c_ in range(2):
                    e_sl = e_ts[h][c_][:, w_ * 256:(w_ + 1) * 256]
                    nc.tensor.matmul(dst[b0:b0 + 32, 0:256],
                                     tvs[w_ * 2 + c_][:, 32 * h:32 * h + 32],
                                     e_sl, start=(c_ == 0), stop=(c_ == 1),
                                     tile_position=(0, b0))
                for c_ in range(2):
                    e_sl = e_ts[h][c_][:, w_ * 256:(w_ + 1) * 256]
                    nc.tensor.matmul(dst[b0:b0 + 32, 256:512],
                                     ones_blk, e_sl,
                                     start=(c_ == 0), stop=(c_ == 1),
                                     tile_position=(0, b0))
            rd = sb_rd.tile([128, 256], F32, tag="rd")
            nc.vector.reciprocal(out=rd, in_=ot[:, 256:512])
            nc.vector.tensor_mul(out=oT_a[:, w_ * 256:(w_ + 1) * 256],
                                 in0=ot[:, 0:256], in1=rd)
            rd2 = sb_rd.tile([64, 256], F32, tag="rd2")
            nc.vector.reciprocal(out=rd2, in_=ot2[:, 256:512])
            nc.vector.tensor_mul(out=oT_b[0:64, w_ * 256:(w_ + 1) * 256],
                                 in0=ot2[:, 0:256], in1=rd2)

        # stage E: proj + residual + LN2 + transpose
        xr_ts = []
        h2T_a = sb_hT.tile([128, 512], F32, tag="h2ta")
        h2T_b = sb_hT.tile([65, 512], F32, tag="h2tb")
        nc.gpsimd.memset(h2T_b[64:65, :], 1.0)
        for i in range(4):
            pp = ps_tr.tile([128, C], F32, tag="tr")
            nc.tensor.matmul(pp, oT_a[:, i * 128:(i + 1) * 128], w_pj_a,
                             start=True, stop=False)
            nc.tensor.matmul(pp, oT_b[:, i * 128:(i + 1) * 128], w_pj_b,
                             start=False, stop=True)
            xr_t = sb_xr.tile([128, C], F32, tag="xr")
            nc.vector.tensor_add(out=xr_t, in0=x_ts[i], in1=pp)
            xr_ts.append(xr_t)
            st = sb_st.tile([128, 6], F32, tag="st")
            nc.vector.bn_stats(out=st, in_=xr_t)
            mv = sb_st.tile([128, 2], F32, tag="mv")
            nc.vector.bn_aggr(out=mv, in_=st)
            sd = sb_st.tile([128, 1], F32, tag="sd")
            nc.scalar.activation(out=sd, in_=mv[:, 1:2], func=AF.Sqrt, bias=eps_t)
            rstd = sb_st.tile([128, 1], F32, tag="rstd")
            nc.vector.reciprocal(out=rstd, in_=sd)
            h2_t = sb_h.tile([128, C], F32, tag="h2")
            nc.gpsimd.tensor_scalar(out=h2_t, in0=xr_t, scalar1=mv[:, 0:1],
                                    scalar2=rstd, op0=mybir.AluOpType.subtract,
                                    op1=mybir.AluOpType.mult)
            ptr = ps_tr.tile([128, 256], F32, tag="tr")
            nc.tensor.transpose(ptr[:, 0:128], h2_t[:, 0:128], ident)
            nc.tensor.transpose(ptr[0:64, 128:256], h2_t[:, 128:192], ident)
            nc.scalar.copy(out=h2T_a[:, i * 128:(i + 1) * 128], in_=ptr[:, 0:128])
            nc.scalar.copy(out=h2T_b[0:64, i * 128:(i + 1) * 128],
                           in_=ptr[0:64, 128:256])

        # stage F: fc1 + gelu
        g_ts = []
        for m in range(6):
            f1 = ps_big.tile([128, 512], F32, tag="big")
            sl = slice(m * 128, (m + 1) * 128)
            nc.tensor.matmul(f1, w_f1_a[:, sl], h2T_a, start=True, stop=False)
            nc.tensor.matmul(f1, w_f1_b[:, sl], h2T_b, start=False, stop=True)
            g_t = sb_g.tile([128, 512], F32, tag="g")
            nc.scalar.activation(out=g_t, in_=f1, func=AF.Gelu)
            g_ts.append(g_t)

        # stage G: fc2 + residual + store
        for i in range(4):
            f2 = ps_tr.tile([128, C], F32, tag="tr")
            for m in range(6):
                nc.tensor.matmul(f2, g_ts[m][:, i * 128:(i + 1) * 128],
                                 w_f2[:, m, :], start=(m == 0),
                                 stop=(m == 5 and not has_fc2_bias))
            if has_fc2_bias:
                nc.tensor.matmul(f2, ones_row[:, i * 128:(i + 1) * 128],
                                 w_f2b, start=False, stop=True)
            fin = sb_fin.tile([128, C], F32, tag="fin")
            nc.vector.tensor_add(out=fin, in0=xr_ts[i], in1=f2)
            nc.sync.dma_start(out=out[tok0 + i * 128: tok0 + (i + 1) * 128, :],
                              in_=fin)


# ---------------------------------------------------------------- entry point

_CACHE = {}


def _get_nc(has_fc2_bias, repeat=1):
    key = (has_fc2_bias, repeat)
    if key not in _CACHE:
        _CACHE[key] = _build_nc(has_fc2_bias, repeat=repeat)
    return _CACHE[key]


def _make_in_maps(x, rpi_sa, attn_mask, folded):
    perm = _window_perm()
    x_win = np.ascontiguousarray(np.asarray(x, np.float32).reshape(H * W, C)[perm.reshape(-1)])
    weights = {k: v for k, v in folded.items() if k != "has_fc2_bias"}
    in_maps = []
    for c in range(NCORES):
        m = dict(weights)
        m["xw"] = np.ascontiguousarray(x_win[c * S * N:(c + 1) * S * N])
        m["cb"] = _CB_CACHE[c]
        in_maps.append(m)
    return in_maps, perm


_CB_CACHE = None


def kernel(x, rpi_sa, attn_mask, norm1_g, norm1_b, qkv_w, qkv_b, rpb_table,
           proj_w, proj_b, norm2_g, norm2_b, fc1_w, fc1_b, fc2_w, fc2_b):
    global _CB_CACHE
    from concourse.bass_utils import run_bass_kernel_spmd

    folded = _fold_weights(norm1_g, norm1_b, qkv_w, qkv_b, proj_w, proj_b,
                           norm2_g, norm2_b, fc1_w, fc1_b, fc2_w, fc2_b)
    _CB_CACHE = [
        _build_cb_core(rpi_sa, rpb_table, attn_mask,
                       list(range(c * S, (c + 1) * S)))
        for c in range(NCORES)
    ]
    nc = _get_nc(folded["has_fc2_bias"])
    in_maps, perm = _make_in_maps(x, rpi_sa, attn_mask, folded)
    res = run_bass_kernel_spmd(nc, in_maps, list(range(NCORES)))
    out_win = np.concatenate([res.results[c]["out"] for c in range(NCORES)], axis=0)
    out_flat = np.empty_like(out_win)
    out_flat[perm.reshape(-1)] = out_win
    return out_flat.reshape(1, H * W, C)
